# revision 32
# baseline (speedup 1.0000x reference)
"""ExpRNN forward on 8 Trainium2 NeuronCores.

Math: Bmat = expm(skew(A)); h_t = modrelu(x_t @ W_in.T + h_{t-1} @ Bmat, b_mod);
out = h_{T-1} @ lin_W.T + lin_b.

When b_mod == 0, modrelu is the identity and the whole network is linear:
    out[b] = sum_t x[b,t,:] @ (W_in.T @ Bmat^(T-1-t) @ lin_W.T) + lin_b
           = X[b, :] @ Kflat + lin_b,   X = inputs.reshape(B, T*D)
which is one memory-bound [B, T*D] @ [T*D, 10] matmul — Kflat is built on the
host from the tiny parameter matrices. Sharding: pure data parallelism over
batch; each of the 8 cores computes its [1024, 4096] @ [4096, 10] slice.

The contraction is split into 32 blocks of 128. To cut HBM traffic X is sent
as float8_e3m4 (1 byte); the PE multiplies fp8 moving data against bf16
stationary weights directly (mixed-dtype matmul, verified exact on HW), so K
keeps bf16 precision and the only extra error is X's e3m4 rounding
(1.35e-2 rel measured, vs the 2e-2 gate).

Slabs of 2 blocks stream on both HWDGE rings (SP = even slabs, ACT = odd) so
aggregate DMA bandwidth is available from t0 and arrival order matches PE
consumption order; per-ring FIFO completion makes rotating completion
semaphores safe. The PE ramps with warmup matmuls (it starts at half rate
until ~3-4us of continuous activity) and then consumes blocks back-to-back,
paced just behind the DMA stream so it never idles long enough for the HAM
power manager to re-throttle it.

For general b_mod the recurrence is evaluated step-by-step on device
(see _recurrent_path).
"""

import numpy as np

B, T, D = 8192, 2048, 2
H, O = 10, 10
N_CORES = 8
B_LOC = B // N_CORES          # 1024 samples per core
KDIM = T * D                  # 4096 contraction length
NCHUNK = KDIM // 128          # 32 contraction blocks of 128

NSLAB = NCHUNK // 2           # 16 slabs of 2 blocks
KA_BLOCKS = 8                 # K for blocks 0..7 rides ahead on the SP ring
N_WARM = 26                   # PE ramp: 128-col warmup matmuls until slab 0

_NC_CACHE = {}


def _expm_skew(A64):
    """expm of skew(A) built from strict upper triangle, float64-exact."""
    S = np.triu(A64, 1)
    S = S - S.T
    w, V = np.linalg.eig(S)           # skew-symmetric => normal, eig is stable
    return (V @ np.diag(np.exp(w)) @ np.linalg.inv(V)).real


def _collapse_weights(A, W_in, lin_W):
    """Kflat [T*D, O] with out = X @ Kflat (valid only when b_mod == 0)."""
    Bm = _expm_skew(A.astype(np.float64))
    W64 = W_in.astype(np.float64)
    L64 = lin_W.astype(np.float64)
    K = np.empty((T, O, D))
    M = L64.copy()                     # lin_W @ (Bm.T)^(T-1-t)
    for t in range(T - 1, -1, -1):
        K[t] = M @ W64
        M = M @ Bm.T
    return np.ascontiguousarray(K.transpose(0, 2, 1).reshape(T * D, O))


# ---------------------------------------------------------------------------
# fast path: b_mod == 0  ->  one big matmul per core
# ---------------------------------------------------------------------------


def _build_linear_nc_raw():
    """Raw-bass version with manual semaphores — avoids TileContext's
    ~8-10us end-of-kernel drain + EVSEM butterfly."""
    import concourse.bass as bass
    from concourse import mybir

    f32 = mybir.dt.float32
    bf16 = mybir.dt.bfloat16
    fp8 = mybir.dt.float8e3
    nc = bass.Bass("TRN2", target_bir_lowering=False, debug=False,
                   num_devices=N_CORES)
    xP8 = nc.dram_tensor("xP8", (128, NCHUNK * B_LOC), fp8,
                         kind="ExternalInput")
    km = nc.dram_tensor("kmat", (128, NCHUNK * O), bf16, kind="ExternalInput")
    out = nc.dram_tensor("out", (O, B_LOC), bf16, kind="ExternalOutput")

    Ident = mybir.ActivationFunctionType.Identity

    import contextlib

    with contextlib.ExitStack() as ctx:
        x8 = ctx.enter_context(
            nc.sbuf_tensor("x8", [128, NCHUNK * B_LOC], fp8))
        ktile = ctx.enter_context(
            nc.sbuf_tensor("ktile", [128, NCHUNK * O], bf16))
        otile = ctx.enter_context(nc.sbuf_tensor("otile", [O, B_LOC], bf16))
        wtile = ctx.enter_context(nc.sbuf_tensor("wtile", [128, 128], bf16))
        ps = ctx.enter_context(nc.psum_tensor("ps", [O, B_LOC], f32))
        ps_warm = ctx.enter_context(nc.psum_tensor("ps_warm", [O, 512], f32))
        # Slab completions rotate over 2 sems per ring: a ring's DGE retires
        # descriptors FIFO, so slab k on a ring completes only after slab k-2
        # on the same sem -> cumulative wait targets are race-free.
        kasem = ctx.enter_context(nc.semaphore("kasem"))
        kbsem = ctx.enter_context(nc.semaphore("kbsem"))
        spsems = [ctx.enter_context(nc.semaphore(f"sp{i}")) for i in range(2)]
        acsems = [ctx.enter_context(nc.semaphore(f"ac{i}")) for i in range(2)]
        wsem = ctx.enter_context(nc.semaphore("wsem"))
        pe_sem = ctx.enter_context(nc.semaphore("pe_sem"))
        e0sem = ctx.enter_context(nc.semaphore("e0sem"))
        e1sem = ctx.enter_context(nc.semaphore("e1sem"))
        osem = ctx.enter_context(nc.semaphore("osem"))
        block = ctx.enter_context(nc.Block())

        def slab_sl(si):
            """x column slice for slab si (blocks 2si, 2si+1)."""
            return slice(2 * si * B_LOC, (2 * si + 2) * B_LOC)

        # slab PAIRS alternate rings (s0,s1 -> SP; s2,s3 -> ACT; ...): the
        # consumer reaches a ring's next pair ~1.7us after its previous one,
        # hiding per-ring delivery latency from the very first slab
        sp_slabs = [si for si in range(NSLAB) if (si // 2) % 2 == 0]
        ac_slabs = [si for si in range(NSLAB) if (si // 2) % 2 == 1]

        def slab_wait(engine, si):
            ring = sp_slabs if (si // 2) % 2 == 0 else ac_slabs
            sems = spsems if (si // 2) % 2 == 0 else acsems
            p = ring.index(si)          # FIFO position within the ring
            engine.wait_ge(sems[p % 2], 16 * (p // 2 + 1))

        @block.sync
        def _(sync):
            # K for blocks 0..7 rides ahead of the even slabs on the SP ring
            sync.dma_start(ktile[:, :KA_BLOCKS * O],
                           km[:, :KA_BLOCKS * O]).then_inc(kasem, 16)
            for p, si in enumerate(sp_slabs):
                sl = slab_sl(si)
                sync.dma_start(x8[:, sl], xP8[:, sl]).then_inc(
                    spsems[p % 2], 16)
            sync.wait_ge(e0sem, 1)
            sync.dma_start(out[:, 0:512],
                           otile[:, 0:512]).then_inc(osem, 16)
            sync.wait_ge(osem, 16)

        @block.scalar
        def _(scalar):
            # rest of K + odd slabs on the ACT ring
            scalar.dma_start(ktile[:, KA_BLOCKS * O:],
                             km[:, KA_BLOCKS * O:]).then_inc(kbsem, 16)
            for p, si in enumerate(ac_slabs):
                sl = slab_sl(si)
                scalar.dma_start(x8[:, sl], xP8[:, sl]).then_inc(
                    acsems[p % 2], 16)
            # preload the Identity act table while DMAs stream, so the
            # tail eviction doesn't pay the ~1.3us ACT_TABLE_LOAD
            scalar.wait_ge(wsem, 1)
            scalar.activation(otile[:, 512:513], wtile[:O, 0:1], Ident)
            # psum bank 1 -> sbuf -> dram, in parallel with SP's bank-0 path
            # (same-engine program order makes the act -> store safe)
            scalar.wait_ge(pe_sem, 1)
            scalar.activation(otile[:, 512:1024], ps[:, 512:1024], Ident)
            scalar.dma_start(out[:, 512:1024],
                             otile[:, 512:1024]).then_inc(e1sem, 16)
            scalar.wait_ge(e1sem, 16)

        @block.tensor
        def _(tensor):
            tensor.wait_ge(wsem, 1)
            for _ in range(N_WARM):
                tensor.matmul(ps_warm[:, :128], wtile[:, :O], wtile[:, :],
                              start=True, stop=True)
            tensor.wait_ge(kasem, 16)
            for ci in range(NCHUNK):
                if ci == KA_BLOCKS:
                    tensor.wait_ge(kbsem, 16)
                if ci % 2 == 0:
                    slab_wait(tensor, ci // 2)
                # last block: finish bank 1 first so the (slower) ACT
                # eviction + store path gets a head start
                order = (1, 0) if ci == NCHUNK - 1 else (0, 1)
                for n in order:
                    i = tensor.matmul(
                        ps[:, n * 512:(n + 1) * 512],
                        ktile[:, ci * O:(ci + 1) * O],
                        x8[:, ci * B_LOC + n * 512:ci * B_LOC + (n + 1) * 512],
                        start=(ci == 0),
                        stop=(ci == NCHUNK - 1),
                    )
                    if ci == NCHUNK - 1:
                        i.then_inc(pe_sem, 1)

        @block.vector
        def _(vector):
            # psum bank 0 -> sbuf on DVE; lin_b is added on the host
            vector.wait_ge(pe_sem, 2)
            vector.tensor_copy(otile[:, 0:512],
                               ps[:, 0:512]).then_inc(e0sem, 1)

        @block.gpsimd
        def _(gpsimd):
            # non-zero, spatially varied warmup data without a DMA round
            # trip (all-zero warmups draw too little power to earn the PE
            # a full HAM allocation)
            gpsimd.iota(wtile[:, :], [[1, 128]], channel_multiplier=3,
                        allow_small_or_imprecise_dtypes=True).then_inc(
                            wsem, 1)

    return nc


def _linear_path(inputs, A, W_in, lin_W, lin_b):
    import ml_dtypes
    from concourse import bass_utils

    if "linear" not in _NC_CACHE:
        _NC_CACHE["linear"] = _build_linear_nc_raw()
    nc = _NC_CACHE["linear"]

    bf16 = ml_dtypes.bfloat16
    fp8 = ml_dtypes.float8_e3m4
    Kflat = _collapse_weights(A, W_in, lin_W).astype(np.float32)
    # kmat[p, ci*O + m] = Kflat[ci*128 + p, m]
    kmat = np.ascontiguousarray(
        Kflat.reshape(NCHUNK, 128, O).transpose(1, 0, 2)
        .reshape(128, NCHUNK * O)).astype(bf16)

    X = inputs.reshape(B, KDIM).astype(np.float32)
    in_maps = []
    for c in range(N_CORES):
        # xP[p, ci*B_LOC + j] = X[c*B_LOC + j, ci*128 + p]
        xc = X[c * B_LOC:(c + 1) * B_LOC]                # [B_LOC, KDIM]
        xP = np.ascontiguousarray(
            xc.reshape(B_LOC, NCHUNK, 128).transpose(2, 1, 0)
            .reshape(128, NCHUNK * B_LOC))
        in_maps.append({"xP8": xP.astype(fp8), "kmat": kmat})

    res = bass_utils.run_bass_kernel_spmd(nc, in_maps, list(range(N_CORES)))
    kernel.last_results = res
    outs = np.concatenate([r["out"].T.astype(np.float32) for r in res.results],
                          axis=0)
    return outs + lin_b.astype(np.float32)[None, :]


# ---------------------------------------------------------------------------
# general path: b_mod != 0  ->  on-device recurrence (exact modrelu)
# ---------------------------------------------------------------------------

G = 8          # batch groups stacked on partitions: G*H = 80 state rows
F = 128        # samples per group = free dim; G*F = B_LOC
NBUF = 8       # ring blocks; x slab DMA covers NBUF//2 steps


def _build_recurrent_nc(T_steps=T):
    """h ring in SBUF [96, NBUF*F]: partitions 0..79 = kron-stacked state,
    80..95 = per-step inputs. One [96->80, F] matmul per step (weights hold
    both the recurrent and input projections), then modrelu as 3 fused ops:
      u = (z abs_max 0) + b      (DVE tensor_scalar, per-partition bias)
      s = Sign(z)                (ACT, parallel)
      h' = max(u, 0) * s         (DVE scalar_tensor_tensor)
    """
    import contextlib

    import concourse.bass as bass
    from concourse import mybir

    f32 = mybir.dt.float32
    nc = bass.Bass("TRN2", target_bir_lowering=False, debug=False,
                   num_devices=N_CORES)
    xarr = nc.dram_tensor("xarr", (2 * G, T_steps * F), f32,
                          kind="ExternalInput")
    cmat = nc.dram_tensor("cmat", (96, 160), f32, kind="ExternalInput")
    bvec = nc.dram_tensor("bvec", (G * H, 1), f32, kind="ExternalInput")
    rout = nc.dram_tensor("rout", (G * H, F), f32, kind="ExternalOutput")

    P = G * H                      # 80 state partitions
    HALF = NBUF // 2 * F           # columns per x slab DMA
    NCYC = T_steps // (NBUF // 2)  # x slab DMA count
    NXS = 8                        # rotating slab sems
    Sign = mybir.ActivationFunctionType.Sign
    Abs = mybir.ActivationFunctionType.Abs
    Alu = mybir.AluOpType

    with contextlib.ExitStack() as ctx:
        R = ctx.enter_context(nc.sbuf_tensor("R", [96, NBUF * F], f32))
        C = ctx.enter_context(nc.sbuf_tensor("C", [96, 160], f32))
        bb = ctx.enter_context(nc.sbuf_tensor("bb", [P, 1], f32))
        sbv = ctx.enter_context(nc.sbuf_tensor("sbv", [P, 2 * F], f32))
        sbu = ctx.enter_context(nc.sbuf_tensor("sbu", [P, 2 * F], f32))
        sbs = ctx.enter_context(nc.sbuf_tensor("sbs", [P, 2 * F], f32))
        osb = ctx.enter_context(nc.sbuf_tensor("osb", [P, F], f32))
        ps = [ctx.enter_context(nc.psum_tensor(f"rps{n}", [P, F], f32))
              for n in range(2)]
        csem = ctx.enter_context(nc.semaphore("csem"))
        bsem = ctx.enter_context(nc.semaphore("bsem"))
        xsems = [ctx.enter_context(nc.semaphore(f"rx{i}"))
                 for i in range(NXS)]
        pe_sem = ctx.enter_context(nc.semaphore("pe_sem"))
        ssem = ctx.enter_context(nc.semaphore("ssem"))
        vsem = ctx.enter_context(nc.semaphore("vsem"))
        usem = ctx.enter_context(nc.semaphore("usem"))
        hsem = ctx.enter_context(nc.semaphore("hsem"))
        ocsem = ctx.enter_context(nc.semaphore("ocsem"))
        osem = ctx.enter_context(nc.semaphore("osem"))
        block = ctx.enter_context(nc.Block())

        def blk(t):
            return (t % NBUF) * F

        @block.sync
        def _(sync):
            sync.dma_start(C[:, :], cmat[:, :]).then_inc(csem, 16)
            for k in range(NCYC):
                if k >= 2:
                    # halves of the ring alternate; cycle k-2's steps must
                    # be consumed before overwriting its x stripe
                    sync.wait_ge(pe_sem, (k - 1) * (NBUF // 2))
                half = (k % 2) * HALF
                sync.dma_start(
                    R[80:96, half:half + HALF],
                    xarr[:, k * HALF:(k + 1) * HALF],
                ).then_inc(xsems[k % NXS], 16)
            sync.wait_ge(ocsem, 1)
            sync.dma_start(rout[:, :], osb[:, :]).then_inc(osem, 16)
            sync.wait_ge(osem, 16)

        @block.tensor
        def _(tensor):
            tensor.wait_ge(csem, 16)
            for t in range(T_steps):
                if t % (NBUF // 2) == 0:
                    k = t // (NBUF // 2)
                    tensor.wait_ge(xsems[k % NXS], 16 * (k // NXS + 1))
                tensor.wait_ge(hsem, t + 1)
                tensor.matmul(ps[t % 2][:, :], C[:, 0:P],
                              R[:, blk(t):blk(t) + F],
                              start=True, stop=True).then_inc(pe_sem, 1)
            # out = lin_W-stack applied to h_T
            tensor.wait_ge(hsem, T_steps + 1)
            tensor.matmul(ps[T_steps % 2][:, :], C[0:P, 80:160],
                          R[0:P, blk(T_steps):blk(T_steps) + F],
                          start=True, stop=True).then_inc(pe_sem, 1)

        @block.scalar
        def _(scalar):
            # a_t = |z|, s_t = sign(z) -- each reads PSUM exactly once
            scalar.dma_start(bb[:, :], bvec[:, :]).then_inc(bsem, 16)
            for t in range(T_steps):
                c = (t % 2) * F
                scalar.wait_ge(pe_sem, t + 1)
                scalar.activation(sbv[:, c:c + F], ps[t % 2][:, :],
                                  Abs).then_inc(vsem, 1)
                scalar.activation(sbs[:, c:c + F], ps[t % 2][:, :],
                                  Sign).then_inc(ssem, 1)

        @block.gpsimd
        def _(gpsimd):
            # u'_t = max(a_t + b, 0)
            gpsimd.memset(R[0:P, 0:F], 0.0).then_inc(hsem, 1)
            gpsimd.wait_ge(bsem, 16)
            for t in range(T_steps):
                c = (t % 2) * F
                gpsimd.wait_ge(vsem, t + 1)
                gpsimd.tensor_scalar(sbu[:, c:c + F], sbv[:, c:c + F],
                                     bb[:, :], 0.0, Alu.add,
                                     Alu.max).then_inc(usem, 1)

        @block.vector
        def _(vector):
            # h'_t = u'_t * s_t (inputs from gpsimd/ACT -> no same-engine
            # RAW on the deep DVE pipeline)
            for t in range(T_steps):
                c = (t % 2) * F
                vector.wait_ge(usem, t + 1)
                vector.wait_ge(ssem, t + 1)
                vector.scalar_tensor_tensor(
                    R[0:P, blk(t + 1):blk(t + 1) + F],
                    sbu[:, c:c + F], 1.0, sbs[:, c:c + F],
                    Alu.mult, Alu.mult).then_inc(hsem, 1)
            vector.wait_ge(pe_sem, T_steps + 1)
            vector.tensor_scalar(osb[:, :], ps[T_steps % 2][:, :],
                                 0.0, None, Alu.add).then_inc(ocsem, 1)

    return nc


def _recurrent_path(inputs, A, W_in, b_mod, lin_W, lin_b):
    import ml_dtypes  # noqa: F401
    from concourse import bass_utils

    if "recurrent" not in _NC_CACHE:
        _NC_CACHE["recurrent"] = _build_recurrent_nc()
    nc = _NC_CACHE["recurrent"]

    Bm = _expm_skew(A.astype(np.float64)).astype(np.float32)
    W32 = W_in.astype(np.float32)
    L32 = lin_W.astype(np.float32)
    cmat = np.zeros((96, 160), np.float32)
    for g in range(G):
        cmat[g * H:(g + 1) * H, g * H:(g + 1) * H] = Bm
        cmat[80 + 2 * g:80 + 2 * g + 2, g * H:(g + 1) * H] = W32.T
        cmat[g * H:(g + 1) * H, 80 + g * H:80 + (g + 1) * H] = L32.T
    bvec = np.ascontiguousarray(
        np.tile(b_mod.astype(np.float32), G).reshape(G * H, 1))

    in_maps = []
    for c in range(N_CORES):
        xc = inputs[c * B_LOC:(c + 1) * B_LOC].astype(np.float32)
        xarr = np.ascontiguousarray(
            xc.reshape(G, F, T, 2).transpose(0, 3, 2, 1).reshape(2 * G, T * F))
        in_maps.append({"xarr": xarr, "cmat": cmat, "bvec": bvec})

    res = bass_utils.run_bass_kernel_spmd(nc, in_maps, list(range(N_CORES)))
    kernel.last_results = res
    outs = []
    for r in res.results:
        ro = r["rout"]                        # [G*H, F]
        outs.append(ro.reshape(G, H, F).transpose(0, 2, 1).reshape(B_LOC, H))
    return np.concatenate(outs, axis=0) + lin_b.astype(np.float32)[None, :]


def kernel(inputs, A, W_in, b_mod, lin_W, lin_b):
    inputs = np.asarray(inputs, np.float32)
    if np.any(np.asarray(b_mod) != 0):
        return _recurrent_path(inputs, A, W_in, b_mod, lin_W, lin_b)
    return _linear_path(inputs, A, W_in, lin_W, lin_b)


# revision 36
# speedup vs baseline: 1.0281x; 1.0281x over previous
"""ExpRNN forward on 8 Trainium2 NeuronCores.

Math: Bmat = expm(skew(A)); h_t = modrelu(x_t @ W_in.T + h_{t-1} @ Bmat, b_mod);
out = h_{T-1} @ lin_W.T + lin_b.

When b_mod == 0, modrelu is the identity and the whole network is linear:
    out[b] = sum_t x[b,t,:] @ (W_in.T @ Bmat^(T-1-t) @ lin_W.T) + lin_b
           = X[b, :] @ Kflat + lin_b,   X = inputs.reshape(B, T*D)
which is one memory-bound [B, T*D] @ [T*D, 10] matmul — Kflat is built on the
host from the tiny parameter matrices. Sharding: pure data parallelism over
batch; each of the 8 cores computes its [1024, 4096] @ [4096, 10] slice.

The contraction is split into 32 blocks of 128. To cut HBM traffic X is sent
as float8_e3m4 (1 byte); the PE multiplies fp8 moving data against bf16
stationary weights directly (mixed-dtype matmul, verified exact on HW), so K
keeps bf16 precision and the only extra error is X's e3m4 rounding
(1.35e-2 rel measured, vs the 2e-2 gate).

Slabs of 2 blocks stream on both HWDGE rings (SP = even slabs, ACT = odd) so
aggregate DMA bandwidth is available from t0 and arrival order matches PE
consumption order; per-ring FIFO completion makes rotating completion
semaphores safe. The PE ramps with warmup matmuls (it starts at half rate
until ~3-4us of continuous activity) and then consumes blocks back-to-back,
paced just behind the DMA stream so it never idles long enough for the HAM
power manager to re-throttle it.

For general b_mod the recurrence is evaluated step-by-step on device
(see _recurrent_path).
"""

import numpy as np

B, T, D = 8192, 2048, 2
H, O = 10, 10
N_CORES = 8
B_LOC = B // N_CORES          # 1024 samples per core
KDIM = T * D                  # 4096 contraction length
NCHUNK = KDIM // 128          # 32 contraction blocks of 128

NSLAB = NCHUNK // 2           # 16 slabs of 2 blocks
N_WARM = 17                   # PE ramp: 128-col warmup matmuls until slab 0

_NC_CACHE = {}


def _expm_skew(A64):
    """expm of skew(A) built from strict upper triangle, float64-exact."""
    S = np.triu(A64, 1)
    S = S - S.T
    w, V = np.linalg.eig(S)           # skew-symmetric => normal, eig is stable
    return (V @ np.diag(np.exp(w)) @ np.linalg.inv(V)).real


def _collapse_weights(A, W_in, lin_W):
    """Kflat [T*D, O] with out = X @ Kflat (valid only when b_mod == 0)."""
    Bm = _expm_skew(A.astype(np.float64))
    W64 = W_in.astype(np.float64)
    L64 = lin_W.astype(np.float64)
    K = np.empty((T, O, D))
    M = L64.copy()                     # lin_W @ (Bm.T)^(T-1-t)
    for t in range(T - 1, -1, -1):
        K[t] = M @ W64
        M = M @ Bm.T
    return np.ascontiguousarray(K.transpose(0, 2, 1).reshape(T * D, O))


# ---------------------------------------------------------------------------
# fast path: b_mod == 0  ->  one big matmul per core
# ---------------------------------------------------------------------------


def _build_linear_nc_raw():
    """Raw-bass version with manual semaphores — avoids TileContext's
    ~8-10us end-of-kernel drain + EVSEM butterfly."""
    import concourse.bass as bass
    from concourse import mybir

    f32 = mybir.dt.float32
    bf16 = mybir.dt.bfloat16
    fp8 = mybir.dt.float8e3
    nc = bass.Bass("TRN2", target_bir_lowering=False, debug=False,
                   num_devices=N_CORES)
    xP8 = nc.dram_tensor("xP8", (128, NCHUNK * B_LOC), fp8,
                         kind="ExternalInput")
    km = nc.dram_tensor("kmat", (128, NCHUNK * O), bf16, kind="ExternalInput")
    out = nc.dram_tensor("out", (O, B_LOC), bf16, kind="ExternalOutput")

    Ident = mybir.ActivationFunctionType.Identity

    import contextlib

    with contextlib.ExitStack() as ctx:
        x8 = ctx.enter_context(
            nc.sbuf_tensor("x8", [128, NCHUNK * B_LOC], fp8))
        ktile = ctx.enter_context(
            nc.sbuf_tensor("ktile", [128, NCHUNK * O], bf16))
        otile = ctx.enter_context(nc.sbuf_tensor("otile", [O, B_LOC], bf16))
        wtile = ctx.enter_context(nc.sbuf_tensor("wtile", [128, 128], bf16))
        ps = ctx.enter_context(nc.psum_tensor("ps", [O, B_LOC], f32))
        ps_warm = ctx.enter_context(nc.psum_tensor("ps_warm", [O, 512], f32))
        # Slab completions rotate over 2 sems per ring: a ring's DGE retires
        # descriptors FIFO, so slab k on a ring completes only after slab k-2
        # on the same sem -> cumulative wait targets are race-free.
        ksem = ctx.enter_context(nc.semaphore("ksem"))
        spsems = [ctx.enter_context(nc.semaphore(f"sp{i}")) for i in range(2)]
        acsems = [ctx.enter_context(nc.semaphore(f"ac{i}")) for i in range(2)]
        wsem = ctx.enter_context(nc.semaphore("wsem"))
        pe_sem = ctx.enter_context(nc.semaphore("pe_sem"))
        e0sem = ctx.enter_context(nc.semaphore("e0sem"))
        e1sem = ctx.enter_context(nc.semaphore("e1sem"))
        osem = ctx.enter_context(nc.semaphore("osem"))
        block = ctx.enter_context(nc.Block())

        def slab_sl(si):
            """x column slice for slab si (blocks 2si, 2si+1)."""
            return slice(2 * si * B_LOC, (2 * si + 2) * B_LOC)

        def slab_wait(engine, si):
            # even slabs ride the ACT ring (its BB starts ~0.9us before
            # Sync's, so slab 0 lands earliest), odd slabs the SP ring
            sems = acsems if si % 2 == 0 else spsems
            p = si // 2                 # FIFO position within the ring
            engine.wait_ge(sems[p % 2], 16 * (p // 2 + 1))

        @block.scalar
        def _(scalar):
            for p, si in enumerate(range(0, NSLAB, 2)):
                sl = slab_sl(si)
                scalar.dma_start(x8[:, sl], xP8[:, sl]).then_inc(
                    acsems[p % 2], 16)
            # preload the Identity act table while DMAs stream, so the
            # tail eviction doesn't pay the ~1.3us ACT_TABLE_LOAD
            scalar.wait_ge(wsem, 1)
            scalar.activation(otile[:, 512:513], wtile[:O, 0:1], Ident)
            # psum bank 1 -> sbuf -> dram, in parallel with Sync's bank-0
            # path (same-engine program order makes the act -> store safe)
            scalar.wait_ge(pe_sem, 1)
            scalar.activation(otile[:, 512:1024], ps[:, 512:1024], Ident)
            scalar.dma_start(out[:, 512:1024],
                             otile[:, 512:1024]).then_inc(e1sem, 16)
            scalar.wait_ge(e1sem, 16)

        @block.sync
        def _(sync):
            # K rides ahead of the odd slabs on the SP ring
            sync.dma_start(ktile[:, :], km[:, :]).then_inc(ksem, 16)
            for p, si in enumerate(range(1, NSLAB, 2)):
                sl = slab_sl(si)
                sync.dma_start(x8[:, sl], xP8[:, sl]).then_inc(
                    spsems[p % 2], 16)
            sync.wait_ge(e0sem, 1)
            sync.dma_start(out[:, 0:512],
                           otile[:, 0:512]).then_inc(osem, 16)
            sync.wait_ge(osem, 16)

        @block.tensor
        def _(tensor):
            tensor.wait_ge(wsem, 1)
            for _ in range(N_WARM):
                tensor.matmul(ps_warm[:, :128], wtile[:, :O], wtile[:, :],
                              start=True, stop=True)
            tensor.wait_ge(ksem, 16)
            for ci in range(NCHUNK):
                if ci % 2 == 0:
                    slab_wait(tensor, ci // 2)
                # last block: finish bank 1 first so the (slower) ACT
                # eviction + store path gets a head start
                order = (1, 0) if ci == NCHUNK - 1 else (0, 1)
                for n in order:
                    i = tensor.matmul(
                        ps[:, n * 512:(n + 1) * 512],
                        ktile[:, ci * O:(ci + 1) * O],
                        x8[:, ci * B_LOC + n * 512:ci * B_LOC + (n + 1) * 512],
                        start=(ci == 0),
                        stop=(ci == NCHUNK - 1),
                    )
                    if ci == NCHUNK - 1:
                        i.then_inc(pe_sem, 1)

        @block.vector
        def _(vector):
            # psum bank 0 -> sbuf on DVE; lin_b is added on the host
            vector.wait_ge(pe_sem, 2)
            vector.tensor_copy(otile[:, 0:512],
                               ps[:, 0:512]).then_inc(e0sem, 1)

        @block.gpsimd
        def _(gpsimd):
            # non-zero, spatially varied warmup data without a DMA round
            # trip (all-zero warmups draw too little power to earn the PE
            # a full HAM allocation)
            gpsimd.iota(wtile[:, :], [[1, 128]], channel_multiplier=3,
                        allow_small_or_imprecise_dtypes=True).then_inc(
                            wsem, 1)

    return nc


def _linear_path(inputs, A, W_in, lin_W, lin_b):
    import ml_dtypes
    from concourse import bass_utils

    if "linear" not in _NC_CACHE:
        _NC_CACHE["linear"] = _build_linear_nc_raw()
    nc = _NC_CACHE["linear"]

    bf16 = ml_dtypes.bfloat16
    fp8 = ml_dtypes.float8_e3m4
    Kflat = _collapse_weights(A, W_in, lin_W).astype(np.float32)
    # kmat[p, ci*O + m] = Kflat[ci*128 + p, m]
    kmat = np.ascontiguousarray(
        Kflat.reshape(NCHUNK, 128, O).transpose(1, 0, 2)
        .reshape(128, NCHUNK * O)).astype(bf16)

    X = inputs.reshape(B, KDIM).astype(np.float32)
    in_maps = []
    for c in range(N_CORES):
        # xP[p, ci*B_LOC + j] = X[c*B_LOC + j, ci*128 + p]
        xc = X[c * B_LOC:(c + 1) * B_LOC]                # [B_LOC, KDIM]
        xP = np.ascontiguousarray(
            xc.reshape(B_LOC, NCHUNK, 128).transpose(2, 1, 0)
            .reshape(128, NCHUNK * B_LOC))
        in_maps.append({"xP8": xP.astype(fp8), "kmat": kmat})

    res = bass_utils.run_bass_kernel_spmd(nc, in_maps, list(range(N_CORES)))
    kernel.last_results = res
    outs = np.concatenate([r["out"].T.astype(np.float32) for r in res.results],
                          axis=0)
    return outs + lin_b.astype(np.float32)[None, :]


# ---------------------------------------------------------------------------
# general path: b_mod != 0  ->  on-device recurrence (exact modrelu)
# ---------------------------------------------------------------------------

G = 8          # batch groups stacked on partitions: G*H = 80 state rows
F = 128        # samples per group = free dim; G*F = B_LOC
NBUF = 8       # ring blocks; x slab DMA covers NBUF//2 steps


def _build_recurrent_nc(T_steps=T):
    """h ring in SBUF [96, NBUF*F]: partitions 0..79 = kron-stacked state,
    80..95 = per-step inputs. One [96->80, F] matmul per step (weights hold
    both the recurrent and input projections), then modrelu as 3 fused ops:
      u = (z abs_max 0) + b      (DVE tensor_scalar, per-partition bias)
      s = Sign(z)                (ACT, parallel)
      h' = max(u, 0) * s         (DVE scalar_tensor_tensor)
    """
    import contextlib

    import concourse.bass as bass
    from concourse import mybir

    f32 = mybir.dt.float32
    nc = bass.Bass("TRN2", target_bir_lowering=False, debug=False,
                   num_devices=N_CORES)
    xarr = nc.dram_tensor("xarr", (2 * G, T_steps * F), f32,
                          kind="ExternalInput")
    cmat = nc.dram_tensor("cmat", (96, 160), f32, kind="ExternalInput")
    bvec = nc.dram_tensor("bvec", (G * H, 1), f32, kind="ExternalInput")
    rout = nc.dram_tensor("rout", (G * H, F), f32, kind="ExternalOutput")

    P = G * H                      # 80 state partitions
    HALF = NBUF // 2 * F           # columns per x slab DMA
    NCYC = T_steps // (NBUF // 2)  # x slab DMA count
    NXS = 8                        # rotating slab sems
    Sign = mybir.ActivationFunctionType.Sign
    Abs = mybir.ActivationFunctionType.Abs
    Alu = mybir.AluOpType

    with contextlib.ExitStack() as ctx:
        R = ctx.enter_context(nc.sbuf_tensor("R", [96, NBUF * F], f32))
        C = ctx.enter_context(nc.sbuf_tensor("C", [96, 160], f32))
        bb = ctx.enter_context(nc.sbuf_tensor("bb", [P, 1], f32))
        sbv = ctx.enter_context(nc.sbuf_tensor("sbv", [P, 2 * F], f32))
        sbu = ctx.enter_context(nc.sbuf_tensor("sbu", [P, 2 * F], f32))
        sbs = ctx.enter_context(nc.sbuf_tensor("sbs", [P, 2 * F], f32))
        osb = ctx.enter_context(nc.sbuf_tensor("osb", [P, F], f32))
        ps = [ctx.enter_context(nc.psum_tensor(f"rps{n}", [P, F], f32))
              for n in range(2)]
        csem = ctx.enter_context(nc.semaphore("csem"))
        bsem = ctx.enter_context(nc.semaphore("bsem"))
        xsems = [ctx.enter_context(nc.semaphore(f"rx{i}"))
                 for i in range(NXS)]
        pe_sem = ctx.enter_context(nc.semaphore("pe_sem"))
        ssem = ctx.enter_context(nc.semaphore("ssem"))
        vsem = ctx.enter_context(nc.semaphore("vsem"))
        usem = ctx.enter_context(nc.semaphore("usem"))
        hsem = ctx.enter_context(nc.semaphore("hsem"))
        ocsem = ctx.enter_context(nc.semaphore("ocsem"))
        osem = ctx.enter_context(nc.semaphore("osem"))
        block = ctx.enter_context(nc.Block())

        def blk(t):
            return (t % NBUF) * F

        @block.sync
        def _(sync):
            sync.dma_start(C[:, :], cmat[:, :]).then_inc(csem, 16)
            for k in range(NCYC):
                if k >= 2:
                    # halves of the ring alternate; cycle k-2's steps must
                    # be consumed before overwriting its x stripe
                    sync.wait_ge(pe_sem, (k - 1) * (NBUF // 2))
                half = (k % 2) * HALF
                sync.dma_start(
                    R[80:96, half:half + HALF],
                    xarr[:, k * HALF:(k + 1) * HALF],
                ).then_inc(xsems[k % NXS], 16)
            sync.wait_ge(ocsem, 1)
            sync.dma_start(rout[:, :], osb[:, :]).then_inc(osem, 16)
            sync.wait_ge(osem, 16)

        @block.tensor
        def _(tensor):
            tensor.wait_ge(csem, 16)
            for t in range(T_steps):
                if t % (NBUF // 2) == 0:
                    k = t // (NBUF // 2)
                    tensor.wait_ge(xsems[k % NXS], 16 * (k // NXS + 1))
                tensor.wait_ge(hsem, t + 1)
                tensor.matmul(ps[t % 2][:, :], C[:, 0:P],
                              R[:, blk(t):blk(t) + F],
                              start=True, stop=True).then_inc(pe_sem, 1)
            # out = lin_W-stack applied to h_T
            tensor.wait_ge(hsem, T_steps + 1)
            tensor.matmul(ps[T_steps % 2][:, :], C[0:P, 80:160],
                          R[0:P, blk(T_steps):blk(T_steps) + F],
                          start=True, stop=True).then_inc(pe_sem, 1)

        @block.scalar
        def _(scalar):
            # a_t = |z|, s_t = sign(z) -- each reads PSUM exactly once
            scalar.dma_start(bb[:, :], bvec[:, :]).then_inc(bsem, 16)
            for t in range(T_steps):
                c = (t % 2) * F
                scalar.wait_ge(pe_sem, t + 1)
                scalar.activation(sbv[:, c:c + F], ps[t % 2][:, :],
                                  Abs).then_inc(vsem, 1)
                scalar.activation(sbs[:, c:c + F], ps[t % 2][:, :],
                                  Sign).then_inc(ssem, 1)

        @block.gpsimd
        def _(gpsimd):
            # u'_t = max(a_t + b, 0)
            gpsimd.memset(R[0:P, 0:F], 0.0).then_inc(hsem, 1)
            gpsimd.wait_ge(bsem, 16)
            for t in range(T_steps):
                c = (t % 2) * F
                gpsimd.wait_ge(vsem, t + 1)
                gpsimd.tensor_scalar(sbu[:, c:c + F], sbv[:, c:c + F],
                                     bb[:, :], 0.0, Alu.add,
                                     Alu.max).then_inc(usem, 1)

        @block.vector
        def _(vector):
            # h'_t = u'_t * s_t (inputs from gpsimd/ACT -> no same-engine
            # RAW on the deep DVE pipeline)
            for t in range(T_steps):
                c = (t % 2) * F
                vector.wait_ge(usem, t + 1)
                vector.wait_ge(ssem, t + 1)
                vector.scalar_tensor_tensor(
                    R[0:P, blk(t + 1):blk(t + 1) + F],
                    sbu[:, c:c + F], 1.0, sbs[:, c:c + F],
                    Alu.mult, Alu.mult).then_inc(hsem, 1)
            vector.wait_ge(pe_sem, T_steps + 1)
            vector.tensor_scalar(osb[:, :], ps[T_steps % 2][:, :],
                                 0.0, None, Alu.add).then_inc(ocsem, 1)

    return nc


def _recurrent_path(inputs, A, W_in, b_mod, lin_W, lin_b):
    import ml_dtypes  # noqa: F401
    from concourse import bass_utils

    if "recurrent" not in _NC_CACHE:
        _NC_CACHE["recurrent"] = _build_recurrent_nc()
    nc = _NC_CACHE["recurrent"]

    Bm = _expm_skew(A.astype(np.float64)).astype(np.float32)
    W32 = W_in.astype(np.float32)
    L32 = lin_W.astype(np.float32)
    cmat = np.zeros((96, 160), np.float32)
    for g in range(G):
        cmat[g * H:(g + 1) * H, g * H:(g + 1) * H] = Bm
        cmat[80 + 2 * g:80 + 2 * g + 2, g * H:(g + 1) * H] = W32.T
        cmat[g * H:(g + 1) * H, 80 + g * H:80 + (g + 1) * H] = L32.T
    bvec = np.ascontiguousarray(
        np.tile(b_mod.astype(np.float32), G).reshape(G * H, 1))

    in_maps = []
    for c in range(N_CORES):
        xc = inputs[c * B_LOC:(c + 1) * B_LOC].astype(np.float32)
        xarr = np.ascontiguousarray(
            xc.reshape(G, F, T, 2).transpose(0, 3, 2, 1).reshape(2 * G, T * F))
        in_maps.append({"xarr": xarr, "cmat": cmat, "bvec": bvec})

    res = bass_utils.run_bass_kernel_spmd(nc, in_maps, list(range(N_CORES)))
    kernel.last_results = res
    outs = []
    for r in res.results:
        ro = r["rout"]                        # [G*H, F]
        outs.append(ro.reshape(G, H, F).transpose(0, 2, 1).reshape(B_LOC, H))
    return np.concatenate(outs, axis=0) + lin_b.astype(np.float32)[None, :]


def kernel(inputs, A, W_in, b_mod, lin_W, lin_b):
    inputs = np.asarray(inputs, np.float32)
    if np.any(np.asarray(b_mod) != 0):
        return _recurrent_path(inputs, A, W_in, b_mod, lin_W, lin_b)
    return _linear_path(inputs, A, W_in, lin_W, lin_b)


# revision 37
# speedup vs baseline: 1.0561x; 1.0272x over previous
"""ExpRNN forward on 8 Trainium2 NeuronCores.

Math: Bmat = expm(skew(A)); h_t = modrelu(x_t @ W_in.T + h_{t-1} @ Bmat, b_mod);
out = h_{T-1} @ lin_W.T + lin_b.

When b_mod == 0, modrelu is the identity and the whole network is linear:
    out[b] = sum_t x[b,t,:] @ (W_in.T @ Bmat^(T-1-t) @ lin_W.T) + lin_b
           = X[b, :] @ Kflat + lin_b,   X = inputs.reshape(B, T*D)
which is one memory-bound [B, T*D] @ [T*D, 10] matmul — Kflat is built on the
host from the tiny parameter matrices. Sharding: pure data parallelism over
batch; each of the 8 cores computes its [1024, 4096] @ [4096, 10] slice.

The contraction is split into 32 blocks of 128. To cut HBM traffic X is sent
as float8_e3m4 (1 byte); the PE multiplies fp8 moving data against bf16
stationary weights directly (mixed-dtype matmul, verified exact on HW), so K
keeps bf16 precision and the only extra error is X's e3m4 rounding
(1.35e-2 rel measured, vs the 2e-2 gate).

Slabs of 2 blocks stream on both HWDGE rings (SP = even slabs, ACT = odd) so
aggregate DMA bandwidth is available from t0 and arrival order matches PE
consumption order; per-ring FIFO completion makes rotating completion
semaphores safe. The PE ramps with warmup matmuls (it starts at half rate
until ~3-4us of continuous activity) and then consumes blocks back-to-back,
paced just behind the DMA stream so it never idles long enough for the HAM
power manager to re-throttle it.

For general b_mod the recurrence is evaluated step-by-step on device
(see _recurrent_path).
"""

import numpy as np

B, T, D = 8192, 2048, 2
H, O = 10, 10
N_CORES = 8
B_LOC = B // N_CORES          # 1024 samples per core
KDIM = T * D                  # 4096 contraction length
NCHUNK = KDIM // 128          # 32 contraction blocks of 128

NSLAB = NCHUNK // 2           # 16 slabs of 2 blocks
N_WARM = 25                   # PE ramp: 128-col warmup matmuls until slab 0

_NC_CACHE = {}


def _expm_skew(A64):
    """expm of skew(A) built from strict upper triangle, float64-exact."""
    S = np.triu(A64, 1)
    S = S - S.T
    w, V = np.linalg.eig(S)           # skew-symmetric => normal, eig is stable
    return (V @ np.diag(np.exp(w)) @ np.linalg.inv(V)).real


def _collapse_weights(A, W_in, lin_W):
    """Kflat [T*D, O] with out = X @ Kflat (valid only when b_mod == 0)."""
    Bm = _expm_skew(A.astype(np.float64))
    W64 = W_in.astype(np.float64)
    L64 = lin_W.astype(np.float64)
    K = np.empty((T, O, D))
    M = L64.copy()                     # lin_W @ (Bm.T)^(T-1-t)
    for t in range(T - 1, -1, -1):
        K[t] = M @ W64
        M = M @ Bm.T
    return np.ascontiguousarray(K.transpose(0, 2, 1).reshape(T * D, O))


# ---------------------------------------------------------------------------
# fast path: b_mod == 0  ->  one big matmul per core
# ---------------------------------------------------------------------------


def _build_linear_nc_raw():
    """Raw-bass version with manual semaphores — avoids TileContext's
    ~8-10us end-of-kernel drain + EVSEM butterfly."""
    import concourse.bass as bass
    from concourse import mybir

    f32 = mybir.dt.float32
    bf16 = mybir.dt.bfloat16
    fp8 = mybir.dt.float8e3
    nc = bass.Bass("TRN2", target_bir_lowering=False, debug=False,
                   num_devices=N_CORES)
    xP8 = nc.dram_tensor("xP8", (128, NCHUNK * B_LOC), fp8,
                         kind="ExternalInput")
    km = nc.dram_tensor("kmat", (128, NCHUNK * O), bf16, kind="ExternalInput")
    out = nc.dram_tensor("out", (O, B_LOC), bf16, kind="ExternalOutput")

    Ident = mybir.ActivationFunctionType.Identity

    import contextlib

    with contextlib.ExitStack() as ctx:
        x8 = ctx.enter_context(
            nc.sbuf_tensor("x8", [128, NCHUNK * B_LOC], fp8))
        ktile = ctx.enter_context(
            nc.sbuf_tensor("ktile", [128, NCHUNK * O], bf16))
        otile = ctx.enter_context(nc.sbuf_tensor("otile", [O, B_LOC], bf16))
        wtile = ctx.enter_context(nc.sbuf_tensor("wtile", [128, 128], bf16))
        ps = ctx.enter_context(nc.psum_tensor("ps", [O, B_LOC], f32))
        ps_warm = ctx.enter_context(nc.psum_tensor("ps_warm", [O, 512], f32))
        # Slab completions rotate over 2 sems per ring: a ring's DGE retires
        # descriptors FIFO, so slab k on a ring completes only after slab k-2
        # on the same sem -> cumulative wait targets are race-free.
        ksem = ctx.enter_context(nc.semaphore("ksem"))
        spsems = [ctx.enter_context(nc.semaphore(f"sp{i}")) for i in range(2)]
        acsems = [ctx.enter_context(nc.semaphore(f"ac{i}")) for i in range(2)]
        wsem = ctx.enter_context(nc.semaphore("wsem"))
        pe_sem = ctx.enter_context(nc.semaphore("pe_sem"))
        e0sem = ctx.enter_context(nc.semaphore("e0sem"))
        e1sem = ctx.enter_context(nc.semaphore("e1sem"))
        osem = ctx.enter_context(nc.semaphore("osem"))
        block = ctx.enter_context(nc.Block())

        def slab_sl(si):
            """x column slice for slab si (blocks 2si, 2si+1)."""
            return slice(2 * si * B_LOC, (2 * si + 2) * B_LOC)

        def slab_wait(engine, si):
            # even slabs ride the ACT ring (its BB starts ~0.9us before
            # Sync's, so slab 0 lands earliest), odd slabs the SP ring
            sems = acsems if si % 2 == 0 else spsems
            p = si // 2                 # FIFO position within the ring
            engine.wait_ge(sems[p % 2], 16 * (p // 2 + 1))

        @block.scalar
        def _(scalar):
            for p, si in enumerate(range(0, NSLAB, 2)):
                sl = slab_sl(si)
                scalar.dma_start(x8[:, sl], xP8[:, sl]).then_inc(
                    acsems[p % 2], 16)
            # preload the Identity act table while DMAs stream, so the
            # tail eviction doesn't pay the ~1.3us ACT_TABLE_LOAD
            scalar.wait_ge(wsem, 1)
            scalar.activation(otile[:, 512:513], wtile[:O, 0:1], Ident)
            # psum bank 1 -> sbuf -> dram, in parallel with Sync's bank-0
            # path (same-engine program order makes the act -> store safe)
            scalar.wait_ge(pe_sem, 1)
            scalar.activation(otile[:, 512:1024], ps[:, 512:1024], Ident)
            scalar.dma_start(out[:, 512:1024],
                             otile[:, 512:1024]).then_inc(e1sem, 16)
            scalar.wait_ge(e1sem, 16)

        @block.sync
        def _(sync):
            # K rides ahead of the odd slabs on the SP ring
            sync.dma_start(ktile[:, :], km[:, :]).then_inc(ksem, 16)
            for p, si in enumerate(range(1, NSLAB, 2)):
                sl = slab_sl(si)
                sync.dma_start(x8[:, sl], xP8[:, sl]).then_inc(
                    spsems[p % 2], 16)
            sync.wait_ge(e0sem, 1)
            sync.dma_start(out[:, 0:512],
                           otile[:, 0:512]).then_inc(osem, 16)
            sync.wait_ge(osem, 16)

        @block.tensor
        def _(tensor):
            tensor.wait_ge(wsem, 1)
            for _ in range(N_WARM):
                tensor.matmul(ps_warm[:, :128], wtile[:, :O], wtile[:, :],
                              start=True, stop=True)
            tensor.wait_ge(ksem, 16)
            for ci in range(NCHUNK):
                if ci % 2 == 0:
                    slab_wait(tensor, ci // 2)
                # last block: finish bank 1 first so the (slower) ACT
                # eviction + store path gets a head start
                order = (1, 0) if ci == NCHUNK - 1 else (0, 1)
                for n in order:
                    i = tensor.matmul(
                        ps[:, n * 512:(n + 1) * 512],
                        ktile[:, ci * O:(ci + 1) * O],
                        x8[:, ci * B_LOC + n * 512:ci * B_LOC + (n + 1) * 512],
                        start=(ci == 0),
                        stop=(ci == NCHUNK - 1),
                    )
                    if ci == NCHUNK - 1:
                        i.then_inc(pe_sem, 1)

        @block.vector
        def _(vector):
            # psum bank 0 -> sbuf on DVE; lin_b is added on the host
            vector.wait_ge(pe_sem, 2)
            vector.tensor_copy(otile[:, 0:512],
                               ps[:, 0:512]).then_inc(e0sem, 1)

        @block.gpsimd
        def _(gpsimd):
            # non-zero, spatially varied warmup data without a DMA round
            # trip (all-zero warmups draw too little power to earn the PE
            # a full HAM allocation)
            gpsimd.iota(wtile[:, :], [[1, 128]], channel_multiplier=3,
                        allow_small_or_imprecise_dtypes=True).then_inc(
                            wsem, 1)

    return nc


def _linear_path(inputs, A, W_in, lin_W, lin_b):
    import ml_dtypes
    from concourse import bass_utils

    if "linear" not in _NC_CACHE:
        _NC_CACHE["linear"] = _build_linear_nc_raw()
    nc = _NC_CACHE["linear"]

    bf16 = ml_dtypes.bfloat16
    fp8 = ml_dtypes.float8_e3m4
    Kflat = _collapse_weights(A, W_in, lin_W).astype(np.float32)
    # kmat[p, ci*O + m] = Kflat[ci*128 + p, m]
    kmat = np.ascontiguousarray(
        Kflat.reshape(NCHUNK, 128, O).transpose(1, 0, 2)
        .reshape(128, NCHUNK * O)).astype(bf16)

    X = inputs.reshape(B, KDIM).astype(np.float32)
    in_maps = []
    for c in range(N_CORES):
        # xP[p, ci*B_LOC + j] = X[c*B_LOC + j, ci*128 + p]
        xc = X[c * B_LOC:(c + 1) * B_LOC]                # [B_LOC, KDIM]
        xP = np.ascontiguousarray(
            xc.reshape(B_LOC, NCHUNK, 128).transpose(2, 1, 0)
            .reshape(128, NCHUNK * B_LOC))
        in_maps.append({"xP8": xP.astype(fp8), "kmat": kmat})

    res = bass_utils.run_bass_kernel_spmd(nc, in_maps, list(range(N_CORES)))
    kernel.last_results = res
    outs = np.concatenate([r["out"].T.astype(np.float32) for r in res.results],
                          axis=0)
    return outs + lin_b.astype(np.float32)[None, :]


# ---------------------------------------------------------------------------
# general path: b_mod != 0  ->  on-device recurrence (exact modrelu)
# ---------------------------------------------------------------------------

G = 8          # batch groups stacked on partitions: G*H = 80 state rows
F = 128        # samples per group = free dim; G*F = B_LOC
NBUF = 8       # ring blocks; x slab DMA covers NBUF//2 steps


def _build_recurrent_nc(T_steps=T):
    """h ring in SBUF [96, NBUF*F]: partitions 0..79 = kron-stacked state,
    80..95 = per-step inputs. One [96->80, F] matmul per step (weights hold
    both the recurrent and input projections), then modrelu as 3 fused ops:
      u = (z abs_max 0) + b      (DVE tensor_scalar, per-partition bias)
      s = Sign(z)                (ACT, parallel)
      h' = max(u, 0) * s         (DVE scalar_tensor_tensor)
    """
    import contextlib

    import concourse.bass as bass
    from concourse import mybir

    f32 = mybir.dt.float32
    nc = bass.Bass("TRN2", target_bir_lowering=False, debug=False,
                   num_devices=N_CORES)
    xarr = nc.dram_tensor("xarr", (2 * G, T_steps * F), f32,
                          kind="ExternalInput")
    cmat = nc.dram_tensor("cmat", (96, 160), f32, kind="ExternalInput")
    bvec = nc.dram_tensor("bvec", (G * H, 1), f32, kind="ExternalInput")
    rout = nc.dram_tensor("rout", (G * H, F), f32, kind="ExternalOutput")

    P = G * H                      # 80 state partitions
    HALF = NBUF // 2 * F           # columns per x slab DMA
    NCYC = T_steps // (NBUF // 2)  # x slab DMA count
    NXS = 8                        # rotating slab sems
    Sign = mybir.ActivationFunctionType.Sign
    Abs = mybir.ActivationFunctionType.Abs
    Alu = mybir.AluOpType

    with contextlib.ExitStack() as ctx:
        R = ctx.enter_context(nc.sbuf_tensor("R", [96, NBUF * F], f32))
        C = ctx.enter_context(nc.sbuf_tensor("C", [96, 160], f32))
        bb = ctx.enter_context(nc.sbuf_tensor("bb", [P, 1], f32))
        sbv = ctx.enter_context(nc.sbuf_tensor("sbv", [P, 2 * F], f32))
        sbu = ctx.enter_context(nc.sbuf_tensor("sbu", [P, 2 * F], f32))
        sbs = ctx.enter_context(nc.sbuf_tensor("sbs", [P, 2 * F], f32))
        osb = ctx.enter_context(nc.sbuf_tensor("osb", [P, F], f32))
        ps = [ctx.enter_context(nc.psum_tensor(f"rps{n}", [P, F], f32))
              for n in range(2)]
        csem = ctx.enter_context(nc.semaphore("csem"))
        bsem = ctx.enter_context(nc.semaphore("bsem"))
        xsems = [ctx.enter_context(nc.semaphore(f"rx{i}"))
                 for i in range(NXS)]
        pe_sem = ctx.enter_context(nc.semaphore("pe_sem"))
        ssem = ctx.enter_context(nc.semaphore("ssem"))
        vsem = ctx.enter_context(nc.semaphore("vsem"))
        usem = ctx.enter_context(nc.semaphore("usem"))
        hsem = ctx.enter_context(nc.semaphore("hsem"))
        ocsem = ctx.enter_context(nc.semaphore("ocsem"))
        osem = ctx.enter_context(nc.semaphore("osem"))
        block = ctx.enter_context(nc.Block())

        def blk(t):
            return (t % NBUF) * F

        @block.sync
        def _(sync):
            sync.dma_start(C[:, :], cmat[:, :]).then_inc(csem, 16)
            for k in range(NCYC):
                if k >= 2:
                    # halves of the ring alternate; cycle k-2's steps must
                    # be consumed before overwriting its x stripe
                    sync.wait_ge(pe_sem, (k - 1) * (NBUF // 2))
                half = (k % 2) * HALF
                sync.dma_start(
                    R[80:96, half:half + HALF],
                    xarr[:, k * HALF:(k + 1) * HALF],
                ).then_inc(xsems[k % NXS], 16)
            sync.wait_ge(ocsem, 1)
            sync.dma_start(rout[:, :], osb[:, :]).then_inc(osem, 16)
            sync.wait_ge(osem, 16)

        @block.tensor
        def _(tensor):
            tensor.wait_ge(csem, 16)
            for t in range(T_steps):
                if t % (NBUF // 2) == 0:
                    k = t // (NBUF // 2)
                    tensor.wait_ge(xsems[k % NXS], 16 * (k // NXS + 1))
                tensor.wait_ge(hsem, t + 1)
                tensor.matmul(ps[t % 2][:, :], C[:, 0:P],
                              R[:, blk(t):blk(t) + F],
                              start=True, stop=True).then_inc(pe_sem, 1)
            # out = lin_W-stack applied to h_T
            tensor.wait_ge(hsem, T_steps + 1)
            tensor.matmul(ps[T_steps % 2][:, :], C[0:P, 80:160],
                          R[0:P, blk(T_steps):blk(T_steps) + F],
                          start=True, stop=True).then_inc(pe_sem, 1)

        @block.scalar
        def _(scalar):
            # a_t = |z|, s_t = sign(z) -- each reads PSUM exactly once
            scalar.dma_start(bb[:, :], bvec[:, :]).then_inc(bsem, 16)
            for t in range(T_steps):
                c = (t % 2) * F
                scalar.wait_ge(pe_sem, t + 1)
                scalar.activation(sbv[:, c:c + F], ps[t % 2][:, :],
                                  Abs).then_inc(vsem, 1)
                scalar.activation(sbs[:, c:c + F], ps[t % 2][:, :],
                                  Sign).then_inc(ssem, 1)

        @block.gpsimd
        def _(gpsimd):
            # u'_t = max(a_t + b, 0)
            gpsimd.memset(R[0:P, 0:F], 0.0).then_inc(hsem, 1)
            gpsimd.wait_ge(bsem, 16)
            for t in range(T_steps):
                c = (t % 2) * F
                gpsimd.wait_ge(vsem, t + 1)
                gpsimd.tensor_scalar(sbu[:, c:c + F], sbv[:, c:c + F],
                                     bb[:, :], 0.0, Alu.add,
                                     Alu.max).then_inc(usem, 1)

        @block.vector
        def _(vector):
            # h'_t = u'_t * s_t (inputs from gpsimd/ACT -> no same-engine
            # RAW on the deep DVE pipeline)
            for t in range(T_steps):
                c = (t % 2) * F
                vector.wait_ge(usem, t + 1)
                vector.wait_ge(ssem, t + 1)
                vector.scalar_tensor_tensor(
                    R[0:P, blk(t + 1):blk(t + 1) + F],
                    sbu[:, c:c + F], 1.0, sbs[:, c:c + F],
                    Alu.mult, Alu.mult).then_inc(hsem, 1)
            vector.wait_ge(pe_sem, T_steps + 1)
            vector.tensor_scalar(osb[:, :], ps[T_steps % 2][:, :],
                                 0.0, None, Alu.add).then_inc(ocsem, 1)

    return nc


def _recurrent_path(inputs, A, W_in, b_mod, lin_W, lin_b):
    import ml_dtypes  # noqa: F401
    from concourse import bass_utils

    if "recurrent" not in _NC_CACHE:
        _NC_CACHE["recurrent"] = _build_recurrent_nc()
    nc = _NC_CACHE["recurrent"]

    Bm = _expm_skew(A.astype(np.float64)).astype(np.float32)
    W32 = W_in.astype(np.float32)
    L32 = lin_W.astype(np.float32)
    cmat = np.zeros((96, 160), np.float32)
    for g in range(G):
        cmat[g * H:(g + 1) * H, g * H:(g + 1) * H] = Bm
        cmat[80 + 2 * g:80 + 2 * g + 2, g * H:(g + 1) * H] = W32.T
        cmat[g * H:(g + 1) * H, 80 + g * H:80 + (g + 1) * H] = L32.T
    bvec = np.ascontiguousarray(
        np.tile(b_mod.astype(np.float32), G).reshape(G * H, 1))

    in_maps = []
    for c in range(N_CORES):
        xc = inputs[c * B_LOC:(c + 1) * B_LOC].astype(np.float32)
        xarr = np.ascontiguousarray(
            xc.reshape(G, F, T, 2).transpose(0, 3, 2, 1).reshape(2 * G, T * F))
        in_maps.append({"xarr": xarr, "cmat": cmat, "bvec": bvec})

    res = bass_utils.run_bass_kernel_spmd(nc, in_maps, list(range(N_CORES)))
    kernel.last_results = res
    outs = []
    for r in res.results:
        ro = r["rout"]                        # [G*H, F]
        outs.append(ro.reshape(G, H, F).transpose(0, 2, 1).reshape(B_LOC, H))
    return np.concatenate(outs, axis=0) + lin_b.astype(np.float32)[None, :]


def kernel(inputs, A, W_in, b_mod, lin_W, lin_b):
    inputs = np.asarray(inputs, np.float32)
    if np.any(np.asarray(b_mod) != 0):
        return _recurrent_path(inputs, A, W_in, b_mod, lin_W, lin_b)
    return _linear_path(inputs, A, W_in, lin_W, lin_b)
